# revision 1
# baseline (speedup 1.0000x reference)
"""Bass/Tile TRN2 kernel for nn_LocalNodeAttentionHead.

Reference computation (per sample b):
    xi = x[:, :, t0]  (center frame)          (C, HW)
    xw = x reshaped                           (C, L)    L = T*H*W
    q  = Wq @ xi + bq                         (CI, HW)
    k  = Wk @ xw + bk                         (CI, L)
    v  = Wv @ xw + bv                         (L, CI)
    S  = q^T k  -> softmax over L             (HW, L)
    y  = softmax(S) @ v                       (CI, HW)
    out = Wo @ y + bo + xi                    (C, HW)

Distribution: pure data-parallel, 4 samples per core on 8 cores.
Algebraic folds: bk drops (softmax shift invariance), bv applied after the
attention sum (rows of P sum to 1), bo folded into the host-prepared residual.
All matmuls run as float32r (full PE rate at free-dim >= 256).
"""

import sys

sys.path.insert(0, "/opt/trn_rl_repo")

import numpy as np

import concourse.bass as bass
import concourse.tile as tile
from concourse import bacc, mybir

F32 = mybir.dt.float32
F32R = mybir.dt.float32r
AF = mybir.ActivationFunctionType

B, C, T, H, W = 32, 512, 9, 14, 14
CI = 512
HWm = H * W  # 196
L = T * HWm  # 1764
CENT = (T // 2) * HWm  # 784, center-frame offset in L
NCORES = 8
BC = B // NCORES  # 4 samples per core

NCH = C // 128  # 4 chunks of the channel dims
LK = 294  # l-chunk for k-proj / scores (6 chunks; even, >=256 for fp32r rate)
NLK = L // LK
LV = 126  # l-chunk for v-proj / P^T / attention sum (14 chunks)
NLV = L // LV
MC = 98  # query-row chunk (2 chunks of HW=196)
NMC = HWm // MC


def build_program():
    nc = bacc.Bacc("TRN2", target_bir_lowering=False, debug=False)

    # all inputs are host-pre-tiled to partition-major layouts so each load
    # is a single fully-contiguous DMA
    x = nc.dram_tensor("x", [BC, 128, NCH, L], F32, kind="ExternalInput").ap()
    xiq = nc.dram_tensor(
        "xiq", [128, NCH, BC, HWm], F32, kind="ExternalInput"
    ).ap()
    xib = nc.dram_tensor("xib", [BC, C, HWm], F32, kind="ExternalInput").ap()
    wqT = nc.dram_tensor("wqT", [128, NCH, CI], F32, kind="ExternalInput").ap()
    wkT = nc.dram_tensor("wkT", [128, NCH, CI], F32, kind="ExternalInput").ap()
    wvT = nc.dram_tensor("wvT", [128, NCH, CI], F32, kind="ExternalInput").ap()
    woT = nc.dram_tensor("woT", [128, NCH, C], F32, kind="ExternalInput").ap()
    bq = nc.dram_tensor("bq", [128, NCH], F32, kind="ExternalInput").ap()
    bv = nc.dram_tensor("bv", [128, NCH], F32, kind="ExternalInput").ap()
    ident = nc.dram_tensor("ident", [128, 128], F32, kind="ExternalInput").ap()
    out = nc.dram_tensor("out", [BC, C, HWm], F32, kind="ExternalOutput").ap()

    with tile.TileContext(nc) as tc:
        with (
            tc.tile_pool(name="const", bufs=1) as const,
            tc.tile_pool(name="sb", bufs=1) as sb,
            tc.tile_pool(name="ps", bufs=8, space="PSUM") as ps,
        ):
            # ---- constants -------------------------------------------------
            # q-path inputs (xi, wq) are DMA'd first so the PE can start on
            # the q projection as early as possible; the remaining weights go
            # on the gpsimd queue to run in parallel.
            xi_sb = const.tile([128, NCH, BC, HWm], F32R)
            nc.sync.dma_start(xi_sb[:], xiq[:].bitcast(F32R))
            wq_sb = const.tile([128, NCH, CI], F32R)
            nc.sync.dma_start(
                wq_sb[:], wqT[:].bitcast(F32R)
            )
            bq_sb = const.tile([128, NCH], F32)
            nc.sync.dma_start(bq_sb[:], bq[:])
            wk_sb = const.tile([128, NCH, CI], F32R)
            nc.gpsimd.dma_start(
                wk_sb[:], wkT[:].bitcast(F32R)
            )
            wv_sb = const.tile([128, NCH, CI], F32R)
            wo_sb = const.tile([128, NCH, C], F32R)
            bv_sb = const.tile([128, NCH], F32)
            id_sb = const.tile([128, 128], F32)
            q_sb = const.tile([128, NCH, BC * HWm], F32R)
            QH = BC * HWm // 2  # 392

            def emit_qproj():
                for ci in range(NCH):
                    for h in range(2):
                        qp = ps.tile([128, QH], F32, tag="ps", name="qp")
                        for j in range(NCH):
                            nc.tensor.matmul(
                                qp[:],
                                wq_sb[:, j, ci * 128 : (ci + 1) * 128],
                                xi_sb[:, j, 2 * h : 2 * h + 2, :],
                                start=(j == 0),
                                stop=(j == NCH - 1),
                            )
                        nc.scalar.activation(
                            q_sb[:, ci, h * QH : (h + 1) * QH],
                            qp[:],
                            AF.Identity,
                            bias=bq_sb[:, ci : ci + 1],
                        )

            # ---- per-sample attention --------------------------------------
            for s in range(BC):
                xw = sb.tile([128, NCH, L], F32R, tag="xw", bufs=2, name="xw")
                # split the load by k-proj l-chunk: the pieces land on parallel
                # HW DMA queues, so the first chunk's projection starts early
                for lc in range(NLK):
                    nc.gpsimd.dma_start(
                        xw[:, :, lc * LK : (lc + 1) * LK],
                        x[s][:, :, lc * LK : (lc + 1) * LK].bitcast(F32R),
                    )
                if s == 0:
                    # bulk constants stream in behind sample 0's window
                    nc.gpsimd.dma_start(
                        wv_sb[:],
                        wvT[:].bitcast(F32R),
                    )
                    nc.gpsimd.dma_start(
                        wo_sb[:],
                        woT[:].bitcast(F32R),
                    )
                    nc.gpsimd.dma_start(bv_sb[:], bv[:])
                    nc.gpsimd.dma_start(id_sb[:], ident[:])

                s_t = []
                cmax = []
                for mc in range(NMC):
                    s_t.append(
                        sb.tile([MC, L], F32, tag=f"s{mc}", bufs=1, name=f"s{mc}")
                    )
                    cmax.append(
                        sb.tile([MC, NLK], F32, tag=f"cm{mc}", bufs=1, name=f"cm{mc}")
                    )

                # k-projection + scores, streamed over l-chunks of 441
                for lc in range(NLK):
                    ksb = sb.tile([128, NCH, LK], F32R, tag="ksb", bufs=2, name="ksb")
                    for ci in range(NCH):
                        kp = ps.tile([128, LK], F32, tag="ps", name="kp")
                        for j in range(NCH):
                            nc.tensor.matmul(
                                kp[:],
                                wk_sb[:, j, ci * 128 : (ci + 1) * 128],
                                xw[:, j, lc * LK : (lc + 1) * LK],
                                start=(j == 0),
                                stop=(j == NCH - 1),
                            )
                        nc.vector.tensor_copy(ksb[:, ci, :], kp[:])
                    if s == 0 and lc == 0:
                        # q inputs arrive on the sync queue while the k chunk
                        # above computes; emit q here so the PE never stalls
                        emit_qproj()
                    for mc in range(NMC):
                        sp = ps.tile([MC, LK], F32, tag="ps", name="sp")
                        for ci in range(NCH):
                            nc.tensor.matmul(
                                sp[:],
                                q_sb[:, ci, s * HWm + mc * MC : s * HWm + (mc + 1) * MC],
                                ksb[:, ci, :],
                                start=(ci == 0),
                                stop=(ci == NCH - 1),
                            )
                        nc.vector.reduce_max(
                            cmax[mc][:, lc : lc + 1], sp[:], axis=mybir.AxisListType.X
                        )
                        nc.scalar.copy(s_t[mc][:, lc * LK : (lc + 1) * LK], sp[:])

                # v-projection for the whole sample (independent of softmax —
                # keeps the PE busy while the exp chain runs)
                v_sb = sb.tile([128, NLV, CI], F32R, tag="vsb", bufs=1, name="v_sb")
                for lc in range(NLV):
                    vp = ps.tile([LV, CI], F32, tag="ps", name="vp")
                    for j in range(NCH):
                        nc.tensor.matmul(
                            vp[:],
                            xw[:, j, lc * LV : (lc + 1) * LV],
                            wv_sb[:, j, :],
                            start=(j == 0),
                            stop=(j == NCH - 1),
                        )
                    nc.vector.tensor_copy(v_sb[0:LV, lc, :], vp[:])

                # softmax over L (rows of s_t); normalization is deferred to
                # the yT copy (attention sum is linear in P)
                rinvs = []
                for mc in range(NMC):
                    negmax = sb.tile([MC, 1], F32, tag="negmax", bufs=2, name="negmax")
                    nc.vector.reduce_max(
                        negmax[:], cmax[mc][:], axis=mybir.AxisListType.X, negate=True
                    )
                    rsum = sb.tile([MC, 1], F32, tag="rsum", bufs=2, name="rsum")
                    nc.scalar.activation(
                        s_t[mc][:],
                        s_t[mc][:],
                        AF.Exp,
                        bias=negmax[:],
                        accum_out=rsum[:],
                    )
                    rinv = sb.tile([MC, 1], F32, tag="rinv", bufs=2, name="rinv")
                    nc.vector.reciprocal(rinv[:], rsum[:])
                    rinvs.append(rinv)

                # P^T (PE transpose) and yT = P @ V, streamed over l-chunks
                yT_ps = [
                    ps.tile([MC, CI], F32, tag="ps", name=f"yT{mc}")
                    for mc in range(NMC)
                ]
                for lc in range(NLV):
                    ptp = ps.tile([LV, HWm], F32, tag="ps", name="ptp")
                    for mc in range(NMC):
                        nc.tensor.transpose(
                            ptp[:, mc * MC : (mc + 1) * MC],
                            s_t[mc][:, lc * LV : (lc + 1) * LV],
                            id_sb[0:MC, 0:MC],
                        )
                    ptsb = sb.tile([128, HWm], F32R, tag="ptsb", bufs=2, name="ptsb")
                    nc.scalar.copy(ptsb[0:LV, :], ptp[:])
                    for mc in range(NMC):
                        nc.tensor.matmul(
                            yT_ps[mc][:],
                            ptsb[0:LV, mc * MC : (mc + 1) * MC],
                            v_sb[0:LV, lc, :],
                            start=(lc == 0),
                            stop=(lc == NLV - 1),
                        )

                # yT -> sbuf (normalized by rinv), PE-transpose to (CI, HW), + bv
                if s % 2 == 0:
                    y2 = sb.tile(
                        [128, NCH, 2 * HWm], F32R, tag="y2", bufs=2, name="y2"
                    )
                ytsb = sb.tile([MC, NMC, CI], F32, tag="ytsb", bufs=1, name="ytsb")
                for mc in range(NMC):
                    nc.vector.tensor_scalar_mul(
                        ytsb[:, mc, :], yT_ps[mc][:], rinvs[mc][:]
                    )
                for dc in range(NCH):
                    ydp = ps.tile([128, HWm], F32, tag="ps", name="ydp")
                    for mc in range(NMC):
                        nc.tensor.transpose(
                            ydp[:, mc * MC : (mc + 1) * MC],
                            ytsb[:, mc, dc * 128 : (dc + 1) * 128],
                            id_sb[0:MC, 0:MC],
                        )
                    nc.vector.tensor_scalar_add(
                        y2[:, dc, (s % 2) * HWm : (s % 2 + 1) * HWm],
                        ydp[:],
                        bv_sb[:, dc : dc + 1],
                    )

                # output projection + residual for the finished pair
                if s % 2 == 1:
                    xib_sb = sb.tile(
                        [128, NCH, 2 * HWm], F32, tag="xib", bufs=1, name="xib"
                    )
                    for ds in range(2):
                        nc.sync.dma_start(
                            xib_sb[:, :, ds * HWm : (ds + 1) * HWm],
                            xib[s - 1 + ds].rearrange("(j p) m -> p j m", p=128),
                        )
                    osb = sb.tile([128, NCH, 2 * HWm], F32, tag="osb", bufs=1, name="osb")
                    for cc in range(NCH):
                        op = ps.tile([128, 2 * HWm], F32, tag="ps", name="op")
                        for dc in range(NCH):
                            nc.tensor.matmul(
                                op[:],
                                wo_sb[:, dc, cc * 128 : (cc + 1) * 128],
                                y2[:, dc, :],
                                start=(dc == 0),
                                stop=(dc == NCH - 1),
                            )
                        nc.vector.tensor_add(osb[:, cc, :], op[:], xib_sb[:, cc, :])
                        # stream each channel block out as soon as it is ready
                        for ds in range(2):
                            nc.sync.dma_start(
                                out[s - 1 + ds].rearrange(
                                    "(j p) m -> j p m", p=128
                                )[cc],
                                osb[:, cc, ds * HWm : (ds + 1) * HWm],
                            )

    nc.compile()
    return nc


_NC = None


def _get_program():
    global _NC
    if _NC is None:
        _NC = build_program()
    return _NC


def make_in_maps(inputs):
    x_window = np.ascontiguousarray(np.asarray(inputs["x_window"], dtype=np.float32))
    Wq = np.asarray(inputs["Wq"], dtype=np.float32)
    bq_ = np.asarray(inputs["bq"], dtype=np.float32)
    Wk = np.asarray(inputs["Wk"], dtype=np.float32)
    Wv = np.asarray(inputs["Wv"], dtype=np.float32)
    bv_ = np.asarray(inputs["bv"], dtype=np.float32)
    Wo = np.asarray(inputs["Wo"], dtype=np.float32)
    bo_ = np.asarray(inputs["bo"], dtype=np.float32)

    xw = x_window.reshape(B, C, L)
    # residual carrier: center frame + output bias
    xib_full = xw[:, :, CENT : CENT + HWm] + bo_[None, :, None]
    xib_full = np.ascontiguousarray(xib_full)

    def tile_w(wt):  # (in, out) -> [128, NCH, out] partition-major
        return np.ascontiguousarray(
            wt.reshape(NCH, 128, -1).transpose(1, 0, 2)
        )

    shared = {
        "wqT": tile_w(Wq.T),
        "wkT": tile_w(Wk.T),
        "wvT": tile_w(Wv.T),
        "woT": tile_w(Wo.T),
        "bq": np.ascontiguousarray(bq_.reshape(NCH, 128).T),
        "bv": np.ascontiguousarray(bv_.reshape(NCH, 128).T),
        "ident": np.eye(128, dtype=np.float32),
    }
    in_maps = []
    for i in range(NCORES):
        m = dict(shared)
        xc = xw[i * BC : (i + 1) * BC]  # (BC, C, L)
        m["x"] = np.ascontiguousarray(
            xc.reshape(BC, NCH, 128, L).transpose(0, 2, 1, 3)
        )
        m["xiq"] = np.ascontiguousarray(
            xc[:, :, CENT : CENT + HWm]
            .reshape(BC, NCH, 128, HWm)
            .transpose(2, 1, 0, 3)
        )
        m["xib"] = np.ascontiguousarray(xib_full[i * BC : (i + 1) * BC])
        in_maps.append(m)
    return in_maps


def run(inputs, trace=False, tmpdir=None):
    from concourse.bass_utils import run_bass_kernel_spmd

    nc = _get_program()
    in_maps = make_in_maps(inputs)
    res = run_bass_kernel_spmd(
        nc, in_maps, core_ids=list(range(NCORES)), trace=trace, tmpdir=tmpdir
    )
    outs = np.stack([res.results[i]["out"] for i in range(NCORES)])  # (8,4,C,HW)
    full = outs.reshape(B, C, HWm).reshape(B, C, 1, H, W).astype(np.float32)
    return full, res


def kernel(**inputs):
    full, _ = run(inputs)
    return full

